# revision 52
# baseline (speedup 1.0000x reference)
"""Multi-head attention (B=4, L=2048, D=1024, H=16) on 8 TRN2 NeuronCores.

Sharding: core c handles batch b=c//2, query half qh=c%2 (1024 query tokens,
all heads, full 2048-key context). K/V projections are duplicated across the
2 cores sharing a batch; no cross-core communication needed.

Per-core dataflow (projections bf16->PSUM fp32; scores in fp8 DoubleRow):
  - Q.T / K.T projections produce fp8e4 tiles [128, tok]: partition 64*a+p
    holds head (2m+a), dk p. The scores matmul runs in DoubleRow perf mode
    (0.5 cyc/row) with the pair dim broadcast (stride 0), which computes
    2*(K.T @ Q); the extra factor 2 is folded into the exp scale (0.0625).
    Projection PSUM is drained on Pool with bias add + fp8 cast.
  - V = x_v @ Wv.T -> VP[j] = [128 keys, 16*(64+1)] bf16 with a per-head
    ones column (AV's 65th output column accumulates softmax denominators).
  - per (h, j): S[128 k, 1024 q] = KT8_h.T @ QT8_h (fp8 DoubleRow, 0.5
    cyc/row); E = exp(S/8) (Act); E *= mask.T (DVE);
    AV[q-tile, 65] += E_qslice.T @ VP65 (bf16, 65-col matmuls, PSUM
    accumulated over j with one start/stop per bank, skip_group_check).
  - per (h, qt): C_q[128 q, 64] = AV[:, :64] / AV[:, 64] (DVE tensor_scalar
    divide); PE-transpose C_q -> [64 dk, 128 q] PSUM; Pool copy into
    CT[m][64*(h%2):, qt*128:].
  - out[q, 1024] = CT.T-chunks @ Wo.T + bo (ones-column bias matmul),
    Pool drain, DMA.
"""

import sys
import functools
from collections import deque

sys.path.insert(0, "/opt/trn_rl_repo")

import numpy as np
import ml_dtypes

BF16NP = ml_dtypes.bfloat16
E4NP = ml_dtypes.float8_e4m3

B, L, D, H, DK = 4, 2048, 1024, 16, 64
NCORES = 8
LQ = L // 2          # query tokens per core
NI = D // 128        # input-dim chunks
NJ = L // 128        # key tiles
NM2 = 4              # fp8 head-group tiles (4 heads each)
NM = D // 128        # dk-dim tiles of C (2 heads each)
VW = H * (DK + 1)    # V tile width incl. per-head ones column (1040)


def _build():
    import concourse.mybir as mybir
    import concourse.tile as tile
    from concourse import bacc

    dt = mybir.dt
    F32, BF, E4 = dt.float32, dt.bfloat16, dt.float8e4
    AF = mybir.ActivationFunctionType
    PM = mybir.MatmulPerfMode

    nc = bacc.Bacc("TRN2", target_bir_lowering=False, debug=False,
                   num_devices=NCORES)

    xq_d = nc.dram_tensor("xq", [NI, 128, LQ], BF, kind="ExternalInput")
    xk_d = nc.dram_tensor("xk", [NI, 128, L], BF, kind="ExternalInput")
    xv_d = nc.dram_tensor("xv", [NI, 128, L], BF, kind="ExternalInput")
    wq_d = nc.dram_tensor("wq", [NI, 128, D], BF, kind="ExternalInput")
    wk_d = nc.dram_tensor("wk", [NI, 128, D], BF, kind="ExternalInput")
    wv_d = nc.dram_tensor("wv", [NI, 128, D], BF, kind="ExternalInput")
    wo_d = nc.dram_tensor("wo", [NI, 128, D], BF, kind="ExternalInput")
    mt_d = nc.dram_tensor("maskt", [NJ, 128, LQ], BF, kind="ExternalInput")
    bq_d = nc.dram_tensor("bqt", [128, NM], F32, kind="ExternalInput")
    bk_d = nc.dram_tensor("bkt", [128, NM], F32, kind="ExternalInput")
    bo_d = nc.dram_tensor("bor", [1, D], BF, kind="ExternalInput")
    id_d = nc.dram_tensor("ident", [128, 128], BF, kind="ExternalInput")
    out_d = nc.dram_tensor("out", [NM, 128, D], BF, kind="ExternalOutput")

    keep = []  # keep single-tile pools' free-closures alive

    def single(shape, dtyp, name):
        t, free = tc.tile(shape, dtyp, name=name)
        keep.append(free)
        return t

    with tile.TileContext(nc) as tc:
        # ---- persistent tiles ----
        QT8 = [single([128, LQ], E4, f"qt8_{m}") for m in range(NM)]
        KT8 = [single([128, L], E4, f"kt8_{m}") for m in range(NM)]
        VP = [single([128, VW], BF, f"vp{j}") for j in range(NJ)]
        CT = [single([128, LQ], BF, f"ct{m}") for m in range(NM)]
        MT = [single([128, LQ], BF, f"mt{j}") for j in range(NJ)]
        bq_sb = single([128, NM], F32, "bq_sb")
        bk_sb = single([128, NM], F32, "bk_sb")
        bo_sb = single([1, D], BF, "bo_sb")
        ones_sb = single([1, 128], BF, "ones_sb")
        id_sb = single([128, 128], BF, "id_sb")

        nc.vector.memset(ones_sb[:], 1.0)
        for j in range(NJ):
            nc.vector.memset(VP[j][:, DK::DK + 1], 1.0)

        with (
            tc.tile_pool(name="wp", bufs=24) as wp,
            tc.tile_pool(name="xqp", bufs=2 * NI) as xqp,
            tc.tile_pool(name="xkp", bufs=NI) as xkp,
            tc.tile_pool(name="xvp", bufs=NI) as xvp,
            tc.tile_pool(name="ep", bufs=6) as ep,
            tc.tile_pool(name="cqp", bufs=8) as cqp,
            tc.tile_pool(name="fp", bufs=2) as fp,
            tc.tile_pool(name="sp", bufs=2, space="PSUM") as sp,
            tc.tile_pool(name="avp", bufs=2, space="PSUM") as avp,
            tc.tile_pool(name="kp", bufs=1, space="PSUM") as kp,
            tc.tile_pool(name="tp", bufs=1, space="PSUM") as tp,
        ):
            # ---------- weight loads ----------
            def load_w(w_dram, nm):
                ws = []
                for i in range(NI):
                    wt = wp.tile([128, D], BF, tag="w", name=f"w{nm}{i}")
                    nc.sync.dma_start(wt[:], w_dram.ap()[i])
                    ws.append(wt)
                return ws

            # critical-path loads first (Q/K weights + biases); use separate
            # engine DMA queues so the startup transfers run in parallel
            wqs = []
            for i in range(NI):
                wt = wp.tile([128, D], BF, tag="w", name=f"wq{i}")
                nc.sync.dma_start(wt[:], wq_d.ap()[i])
                wqs.append(wt)
            wks = []
            for i in range(NI):
                kt = wp.tile([128, D], BF, tag="w", name=f"wk{i}")
                nc.scalar.dma_start(kt[:], wk_d.ap()[i])
                wks.append(kt)
            nc.sync.dma_start(bq_sb[:], bq_d.ap())
            nc.sync.dma_start(bk_sb[:], bk_d.ap())

            # xq tiles (resident through all Q-proj units), on the Act queue
            # (hardware DGE; the gpsimd queue would burn Pool engine time)
            xqs = {}
            for i in range(NI):
                for c in range(LQ // 512):
                    xt = xqp.tile([128, 512], BF, tag="xq", name=f"xq{c}_{i}")
                    nc.scalar.dma_start(xt[:], xq_d.ap()[i, :, c * 512:(c + 1) * 512])
                    xqs[(c, i)] = xt

            # ---------- unit generators (each unit ~1.7us of PE) ----------
            def q_unit(c, m):
                ps = sp.tile([128, 512], F32, tag="s", name=f"psq{c}{m}")
                for i in range(NI):
                    nc.tensor.matmul(ps[:], wqs[i][:, m * 128:(m + 1) * 128],
                                     xqs[(c, i)][:],
                                     start=(i == 0), stop=(i == NI - 1))
                nc.gpsimd.tensor_scalar_add(
                    QT8[m][:, c * 512:(c + 1) * 512], ps[:],
                    bq_sb[:, m:m + 1])

            def load_x(x_dram, c, xs_box, nm, pool, tag):
                # one step of the filler queue: emit the 8 x-tile DMAs for
                # chunk c (ring-ordered with the units that consume them)
                def step():
                    xs = []
                    for i in range(NI):
                        xt = pool.tile([128, 512], BF, tag=tag,
                                       name=f"x{nm}{c}_{i}")
                        nc.sync.dma_start(
                            xt[:], x_dram.ap()[i, :, c * 512:(c + 1) * 512])
                        xs.append(xt)
                    xs_box[(nm, c)] = xs
                return step

            def k_seq(cs, ms, xs_box, nm):
                steps = []
                for c in cs:
                    steps.append(load_x(xk_d, c, xs_box, nm, xkp, "xk"))
                    for m in ms:
                        def unit(c=c, m=m):
                            xs = xs_box[(nm, c)]
                            ps = kp.tile([128, 512], F32, tag="k",
                                         name=f"psk{nm}{c}{m}")
                            for i in range(NI):
                                nc.tensor.matmul(
                                    ps[:], wks[i][:, m * 128:(m + 1) * 128],
                                    xs[i][:],
                                    start=(i == 0), stop=(i == NI - 1))
                            nc.gpsimd.tensor_scalar_add(
                                KT8[m][:, c * 512:(c + 1) * 512],
                                ps[:], bk_sb[:, m:m + 1])
                        steps.append(unit)
                return steps

            def v_seq(xs_box):
                # each step tagged: real V units set v_frontier when emitted
                steps = []
                for c in range(L // 512):
                    steps.append(load_x(xv_d, c, xs_box, "v", xvp, "xv"))
                    for jj in range(4):
                        j = c * 4 + jj
                        for half in range(2):
                            def unit(c=c, j=j, jj=jj, half=half):
                                xs = xs_box[("v", c)]
                                wvs = wvs_box["wv"]
                                ps = kp.tile([128, 512], F32, tag="k",
                                             name=f"psv{j}{half}")
                                hs = slice(half * 512, half * 512 + 512)
                                for i in range(NI):
                                    nc.tensor.matmul(
                                        ps[:], xs[i][:, jj * 128:(jj + 1) * 128],
                                        wvs[i][:, hs],
                                        start=(i == 0), stop=(i == NI - 1))
                                dst = VP[j][:].rearrange(
                                    "p (h w) -> p h w", w=DK + 1)[
                                    :, half * 8:(half + 1) * 8, 0:DK]
                                src = ps[:].rearrange("p (h w) -> p h w", w=DK)
                                nc.gpsimd.tensor_copy(dst, src)
                                if half == 1:
                                    v_frontier[0] = j + 1
                            steps.append(unit)
                return steps

            # ---------- build filler queues ----------
            v_frontier = [0]  # number of VP tiles whose producing unit was emitted
            wvs_box = {}
            wos_box = {}
            xs_box = {}

            def load_wv():
                wvs_box["wv"] = load_w(wv_d, "v")

            def load_wo():
                wos_box["wo"] = load_w(wo_d, "o")

            v_queue = deque([load_wv] + v_seq(xs_box))

            # non-V fillers in deadline order:
            # K pass A c1..c3 (h0-h7 need m 0..3 as c chunks land), then the
            # remaining Q units (frees the wq pool slots early), K pass B
            # (m 4..7, by h8 = unit 128), wo load.
            other = deque()
            other.extend(k_seq([1, 2, 3], [0, 1, 2, 3], xs_box, "a"))  # 15
            for m in range(2, NM):
                for c in (0, 1):
                    other.append(functools.partial(q_unit, c, m))       # 12
            kB = k_seq([0, 1, 2, 3], [4, 5, 6, 7], xs_box, "b")        # 20
            other.extend(kB)
            other.append(load_wo)
            len_other = len(other)

            # ---------- direct pre-phase ----------
            # shortest path to the first exp: Q m0 (both chunks), K c0 m0
            ka0 = k_seq([0], [0, 1, 2, 3], xs_box, "a0")
            q_unit(0, 0)
            q_unit(1, 0)
            ka0[0]()     # xk c0 loads
            ka0[1]()     # K c0 m0
            # early mask loads (needed from the first mask-mul on)
            for j in range(4):
                nc.scalar.dma_start(MT[j][:], mt_d.ap()[j])
            q_unit(0, 1)
            q_unit(1, 1)
            for u in ka0[2:]:
                u()
            for j in range(4, NJ):
                nc.scalar.dma_start(MT[j][:], mt_d.ap()[j])
            nc.scalar.dma_start(bo_sb[:], bo_d.ap())
            nc.scalar.dma_start(id_sb[:], id_d.ap())

            # ---------- attention ----------
            pend_av = deque()   # (h, j, e_tile) waiting for VP[j]
            av_tiles = {}       # h -> (av0, av1)
            av_count = {}       # h -> emitted AV units (of 16)
            done_h = [0]        # heads fully AV'd + div'd

            def emit_av(h, j, e):
                av0, av1 = av_tiles[h]
                cnt = av_count[h]
                for qt in range(8):
                    avt = av0 if qt < 4 else av1
                    sl = slice(65 * (qt % 4), 65 * (qt % 4) + 65)
                    nc.tensor.matmul(
                        avt[:, sl], e[:, qt * 128:(qt + 1) * 128],
                        VP[j][:, 65 * h:65 * h + 65],
                        start=(cnt == 0 and qt % 4 == 0),
                        stop=(cnt == 15 and qt % 4 == 3),
                        skip_group_check=True)
                av_count[h] = cnt + 1

            pend_fin = deque()  # deferred transpose steps (h, qt, cq, tr)

            def finish_h(h):
                # emit all 8 divs now (DVE, off PE's critical path); defer
                # the PE transposes into later units so they never block the
                # in-order PE stream while the divs complete.
                av0, av1 = av_tiles[h]
                m, off = h // 2, DK * (h % 2)
                tr = tp.tile([128, LQ], BF, tag="t", name=f"tr{h}")
                for qt in range(8):
                    avt = av0 if qt < 4 else av1
                    sl0 = 65 * (qt % 4)
                    cq = cqp.tile([128, DK], BF, tag="cq", name=f"cq{h}_{qt}")
                    nc.vector.tensor_scalar(
                        cq[:], avt[:, sl0:sl0 + DK],
                        avt[:, sl0 + DK:sl0 + DK + 1], None,
                        op0=mybir.AluOpType.divide)
                    pend_fin.append((h, m, off, qt, cq, tr))
                del av_tiles[h]

            def pop_fin(nmax):
                for _ in range(nmax):
                    if not pend_fin:
                        return
                    h, m, off, qt, cq, tr = pend_fin.popleft()
                    nc.tensor.matmul(
                        tr[off:off + DK, qt * 128:(qt + 1) * 128],
                        cq[:], id_sb[:],
                        is_transpose=True, start=True, stop=True)
                    if qt == 7:
                        nc.gpsimd.tensor_copy(CT[m][off:off + DK, :],
                                              tr[off:off + DK, :])

            def drain_pending():
                while pend_av and pend_av[0][1] < v_frontier[0]:
                    h, j, e = pend_av.popleft()
                    emit_av(h, j, e)
                    if av_count[h] == NJ:
                        finish_h(h)
                        done_h[0] += 1

            unit_idx = [0]
            other_pops = [0]

            def pace_fillers():
                u = unit_idx[0]
                # target pops of `other`: 1/unit for first 15 (K pass A incl
                # loads), then 1 per 3 units
                target = min(len_other, u + 1 if u < 15 else 15 + (u - 15) // 3 + 1)
                while other and other_pops[0] < target:
                    other.popleft()()
                    other_pops[0] += 1
                # V: one unit per attention unit from the start (frees the
                # E-park as early as possible), plus demand-driven pulls when
                # pending AVs stack up
                if v_queue:
                    v_queue.popleft()()
                    drain_pending()
                while pend_av and len(pend_av) >= 4 and v_queue:
                    v_queue.popleft()()
                    drain_pending()

            for h in range(H):
                m, a = h // 2, h % 2
                av_tiles[h] = (
                    avp.tile([128, 260], F32, tag="av", name=f"av{h}_0"),
                    avp.tile([128, 260], F32, tag="av", name=f"av{h}_1"),
                )
                av_count[h] = 0
                for j in range(NJ):
                    s = sp.tile([128, LQ], F32, tag="s", name=f"s{h}_{j}")
                    for half in range(2):
                        lhsT = KT8[m][64 * a:64 * a + 64,
                                      j * 128:(j + 1) * 128].unsqueeze(
                            1).broadcast_to([64, 2, 128])
                        rhs = QT8[m][64 * a:64 * a + 64,
                                     half * 512:half * 512 + 512].unsqueeze(
                            1).broadcast_to([64, 2, 512])
                        nc.tensor.matmul(
                            s[:, half * 512:half * 512 + 512], lhsT, rhs,
                            start=True, stop=True, perf_mode=PM.DoubleRow)
                    e = ep.tile([128, LQ], BF, tag="e", name=f"e{h}_{j}")
                    # stride-0 DoubleRow computes 2*S, so halve the scale
                    nc.scalar.activation(e[:], s[:], AF.Exp, scale=0.0625)
                    nc.vector.tensor_mul(e[:], e[:], MT[j][:])
                    # drain BEFORE pushing the current unit: AV(h,j) is
                    # emitted at unit j+1, so the PE stream never waits on
                    # this unit's exp/mask chain (software pipelining lag 1)
                    drain_pending()
                    pop_fin(2)
                    pend_av.append((h, j, e))
                    unit_idx[0] += 1
                    pace_fillers()

            while v_queue:
                v_queue.popleft()()
            drain_pending()
            pop_fin(len(pend_fin))
            while other:
                other.popleft()()
            assert not pend_av and done_h[0] == H, (len(pend_av), done_h[0])

            # ---------- output projection ----------
            wos = wos_box["wo"]
            for t in range(NM):
                po = sp.tile([128, D], F32, tag="s", name=f"po{t}")
                for half in range(2):
                    hs = slice(half * 512, half * 512 + 512)
                    for cc in range(NI):
                        nc.tensor.matmul(
                            po[:, hs], CT[cc][:, t * 128:(t + 1) * 128],
                            wos[cc][:, hs], start=(cc == 0), stop=False)
                for half in range(2):
                    hs = slice(half * 512, half * 512 + 512)
                    nc.tensor.matmul(po[:, hs], ones_sb[:], bo_sb[:, hs],
                                     start=False, stop=True)
                f = fp.tile([128, D], BF, tag="f", name=f"f{t}")
                nc.gpsimd.tensor_copy(f[:], po[:])
                nc.sync.dma_start(out_d.ap()[t], f[:])

    nc.compile()
    nc._keep_tile_frees = keep
    return nc


@functools.lru_cache(maxsize=1)
def _built():
    return _build()


def _prep_core(c, q, k, v, mask01T, wqt, wkt, wvt, wot, bqt, bkt, bor, ident):
    b, qh = c // 2, c % 2
    qs = slice(qh * LQ, (qh + 1) * LQ)
    xq = np.ascontiguousarray(q[b, qs, :].T).astype(BF16NP).reshape(NI, 128, LQ)
    xk = np.ascontiguousarray(k[b].T).astype(BF16NP).reshape(NI, 128, L)
    xv = np.ascontiguousarray(v[b].T).astype(BF16NP).reshape(NI, 128, L)
    maskt = np.ascontiguousarray(mask01T[:, qs]).reshape(NJ, 128, LQ)
    return {
        "xq": xq, "xk": xk, "xv": xv,
        "wq": wqt, "wk": wkt, "wv": wvt, "wo": wot,
        "maskt": maskt, "bqt": bqt, "bkt": bkt, "bor": bor, "ident": ident,
    }


def kernel(q, k, v, attn_mask, Wq, bq, Wk, bk, Wv, bv, Wo, bo):
    from concourse import bass_utils

    nc = _built()

    q = np.asarray(q, np.float32)
    k = np.asarray(k, np.float32)
    v = np.asarray(v, np.float32)
    wqt = np.ascontiguousarray(np.asarray(Wq, np.float32).T).astype(BF16NP).reshape(NI, 128, D)
    wkt = np.ascontiguousarray(np.asarray(Wk, np.float32).T).astype(BF16NP).reshape(NI, 128, D)
    wvt = np.ascontiguousarray(np.asarray(Wv, np.float32).T).astype(BF16NP).reshape(NI, 128, D)
    wot = np.ascontiguousarray(np.asarray(Wo, np.float32).T).astype(BF16NP).reshape(NI, 128, D)
    mask01T = np.ascontiguousarray((np.asarray(attn_mask)[0, 0] != 0).T.astype(BF16NP))
    bqt = np.ascontiguousarray(np.asarray(bq, np.float32).reshape(NM, 128).T)
    bkt = np.ascontiguousarray(np.asarray(bk, np.float32).reshape(NM, 128).T)
    bo_eff = np.asarray(bo, np.float32) + np.asarray(Wo, np.float32) @ np.asarray(bv, np.float32)
    bor = bo_eff.astype(BF16NP).reshape(1, D)
    ident = np.eye(128, dtype=np.float32).astype(BF16NP)

    in_maps = [
        _prep_core(c, q, k, v, mask01T, wqt, wkt, wvt, wot, bqt, bkt, bor, ident)
        for c in range(NCORES)
    ]
    res = bass_utils.run_bass_kernel_spmd(nc, in_maps, core_ids=list(range(NCORES)))

    out = np.empty((B, L, D), np.float32)
    for c in range(NCORES):
        b, qh = c // 2, c % 2
        out[b, qh * LQ:(qh + 1) * LQ, :] = (
            res.results[c]["out"].astype(np.float32).reshape(LQ, D))
    return out


# revision 55
# speedup vs baseline: 1.0125x; 1.0125x over previous
"""Multi-head attention (B=4, L=2048, D=1024, H=16) on 8 TRN2 NeuronCores.

Sharding: core c handles batch b=c//2, query half qh=c%2 (1024 query tokens,
all heads, full 2048-key context). K/V projections are duplicated across the
2 cores sharing a batch; no cross-core communication needed.

Per-core dataflow (projections bf16->PSUM fp32; scores in fp8 DoubleRow):
  - Q.T / K.T projections produce fp8e4 tiles [128, tok]: partition 64*a+p
    holds head (2m+a), dk p. The scores matmul runs in DoubleRow perf mode
    (0.5 cyc/row) with the pair dim broadcast (stride 0), which computes
    2*(K.T @ Q); the extra factor 2 is folded into the exp scale (0.0625).
    Projection PSUM is drained on Pool with bias add + fp8 cast.
  - V = x_v @ Wv.T -> VP[j] = [128 keys, 16*(64+1)] bf16 with a per-head
    ones column (AV's 65th output column accumulates softmax denominators).
  - per (h, j): S[128 k, 1024 q] = KT8_h.T @ QT8_h (fp8 DoubleRow, 0.5
    cyc/row); E = exp(S/8) (Act); E *= mask.T (DVE);
    AV[q-tile, 65] += E_qslice.T @ VP65 (bf16, 65-col matmuls, PSUM
    accumulated over j with one start/stop per bank, skip_group_check).
  - per (h, qt): C_q[128 q, 64] = AV[:, :64] / AV[:, 64] (DVE tensor_scalar
    divide); PE-transpose C_q -> [64 dk, 128 q] PSUM; Pool copy into
    CT[m][64*(h%2):, qt*128:].
  - out[q, 1024] = CT.T-chunks @ Wo.T + bo (ones-column bias matmul),
    Pool drain, DMA.
"""

import sys
import functools
from collections import deque

sys.path.insert(0, "/opt/trn_rl_repo")

import numpy as np
import ml_dtypes

BF16NP = ml_dtypes.bfloat16
E4NP = ml_dtypes.float8_e4m3

B, L, D, H, DK = 4, 2048, 1024, 16, 64
NCORES = 8
LQ = L // 2          # query tokens per core
NI = D // 128        # input-dim chunks
NJ = L // 128        # key tiles
NM2 = 4              # fp8 head-group tiles (4 heads each)
NM = D // 128        # dk-dim tiles of C (2 heads each)
VW = H * (DK + 1)    # V tile width incl. per-head ones column (1040)


def _build():
    import concourse.mybir as mybir
    import concourse.tile as tile
    from concourse import bacc

    dt = mybir.dt
    F32, BF, E4 = dt.float32, dt.bfloat16, dt.float8e4
    AF = mybir.ActivationFunctionType
    PM = mybir.MatmulPerfMode

    nc = bacc.Bacc("TRN2", target_bir_lowering=False, debug=False,
                   num_devices=NCORES)

    xq_d = nc.dram_tensor("xq", [NI, 128, LQ], BF, kind="ExternalInput")
    xk_d = nc.dram_tensor("xk", [NI, 128, L], BF, kind="ExternalInput")
    xv_d = nc.dram_tensor("xv", [NI, 128, L], BF, kind="ExternalInput")
    wq_d = nc.dram_tensor("wq", [NI, 128, D], BF, kind="ExternalInput")
    wk_d = nc.dram_tensor("wk", [NI, 128, D], BF, kind="ExternalInput")
    wv_d = nc.dram_tensor("wv", [NI, 128, D], BF, kind="ExternalInput")
    wo_d = nc.dram_tensor("wo", [NI, 128, D], BF, kind="ExternalInput")
    mt_d = nc.dram_tensor("maskt", [NJ, 128, LQ], BF, kind="ExternalInput")
    bq_d = nc.dram_tensor("bqt", [128, NM], F32, kind="ExternalInput")
    bk_d = nc.dram_tensor("bkt", [128, NM], F32, kind="ExternalInput")
    bo_d = nc.dram_tensor("bor", [1, D], BF, kind="ExternalInput")
    id_d = nc.dram_tensor("ident", [128, 128], BF, kind="ExternalInput")
    out_d = nc.dram_tensor("out", [NM, 128, D], BF, kind="ExternalOutput")

    keep = []  # keep single-tile pools' free-closures alive

    def single(shape, dtyp, name):
        t, free = tc.tile(shape, dtyp, name=name)
        keep.append(free)
        return t

    with tile.TileContext(nc) as tc:
        # ---- persistent tiles ----
        QT8 = [single([128, LQ], E4, f"qt8_{m}") for m in range(NM)]
        KT8 = [single([128, L], E4, f"kt8_{m}") for m in range(NM)]
        VP = [single([128, VW], BF, f"vp{j}") for j in range(NJ)]
        CT = [single([128, LQ], BF, f"ct{m}") for m in range(NM)]
        MT = [single([128, LQ], BF, f"mt{j}") for j in range(NJ)]
        bq_sb = single([128, NM], F32, "bq_sb")
        bk_sb = single([128, NM], F32, "bk_sb")
        bo_sb = single([1, D], BF, "bo_sb")
        ones_sb = single([1, 128], BF, "ones_sb")
        id_sb = single([128, 128], BF, "id_sb")

        nc.vector.memset(ones_sb[:], 1.0)
        for j in range(NJ):
            nc.vector.memset(VP[j][:, DK::DK + 1], 1.0)

        with (
            tc.tile_pool(name="wp", bufs=24) as wp,
            tc.tile_pool(name="xqp", bufs=2 * NI) as xqp,
            tc.tile_pool(name="xkp", bufs=NI) as xkp,
            tc.tile_pool(name="xvp", bufs=NI) as xvp,
            tc.tile_pool(name="ep", bufs=6) as ep,
            tc.tile_pool(name="cqp", bufs=8) as cqp,
            tc.tile_pool(name="fp", bufs=2) as fp,
            tc.tile_pool(name="sp", bufs=2, space="PSUM") as sp,
            tc.tile_pool(name="avp", bufs=2, space="PSUM") as avp,
            tc.tile_pool(name="kp", bufs=1, space="PSUM") as kp,
            tc.tile_pool(name="tp", bufs=1, space="PSUM") as tp,
        ):
            # ---------- weight loads ----------
            def load_w(w_dram, nm):
                ws = []
                for i in range(NI):
                    wt = wp.tile([128, D], BF, tag="w", name=f"w{nm}{i}")
                    nc.sync.dma_start(wt[:], w_dram.ap()[i])
                    ws.append(wt)
                return ws

            # critical-path loads first (Q/K weights + biases); use separate
            # engine DMA queues so the startup transfers run in parallel
            wqs = []
            for i in range(NI):
                wt = wp.tile([128, D], BF, tag="w", name=f"wq{i}")
                nc.sync.dma_start(wt[:], wq_d.ap()[i])
                wqs.append(wt)
            wks = []
            for i in range(NI):
                kt = wp.tile([128, D], BF, tag="w", name=f"wk{i}")
                nc.sync.dma_start(kt[:], wk_d.ap()[i])
                wks.append(kt)
            nc.sync.dma_start(bq_sb[:], bq_d.ap())
            nc.sync.dma_start(bk_sb[:], bk_d.ap())

            # xq tiles (resident through all Q-proj units)
            xqs = {}
            for c in range(LQ // 512):
                for i in range(NI):
                    xt = xqp.tile([128, 512], BF, tag="xq", name=f"xq{c}_{i}")
                    nc.sync.dma_start(xt[:], xq_d.ap()[i, :, c * 512:(c + 1) * 512])
                    xqs[(c, i)] = xt

            # ---------- unit generators (each unit ~1.7us of PE) ----------
            def q_unit(c, m):
                ps = sp.tile([128, 512], F32, tag="s", name=f"psq{c}{m}")
                for i in range(NI):
                    nc.tensor.matmul(ps[:], wqs[i][:, m * 128:(m + 1) * 128],
                                     xqs[(c, i)][:],
                                     start=(i == 0), stop=(i == NI - 1))
                nc.gpsimd.tensor_scalar_add(
                    QT8[m][:, c * 512:(c + 1) * 512], ps[:],
                    bq_sb[:, m:m + 1])

            def load_x(x_dram, c, xs_box, nm, pool, tag):
                # one step of the filler queue: emit the 8 x-tile DMAs for
                # chunk c (ring-ordered with the units that consume them)
                def step():
                    xs = []
                    for i in range(NI):
                        xt = pool.tile([128, 512], BF, tag=tag,
                                       name=f"x{nm}{c}_{i}")
                        nc.sync.dma_start(
                            xt[:], x_dram.ap()[i, :, c * 512:(c + 1) * 512])
                        xs.append(xt)
                    xs_box[(nm, c)] = xs
                return step

            def k_seq(cs, ms, xs_box, nm):
                steps = []
                for c in cs:
                    steps.append(load_x(xk_d, c, xs_box, nm, xkp, "xk"))
                    for m in ms:
                        def unit(c=c, m=m):
                            xs = xs_box[(nm, c)]
                            ps = kp.tile([128, 512], F32, tag="k",
                                         name=f"psk{nm}{c}{m}")
                            for i in range(NI):
                                nc.tensor.matmul(
                                    ps[:], wks[i][:, m * 128:(m + 1) * 128],
                                    xs[i][:],
                                    start=(i == 0), stop=(i == NI - 1))
                            nc.gpsimd.tensor_scalar_add(
                                KT8[m][:, c * 512:(c + 1) * 512],
                                ps[:], bk_sb[:, m:m + 1])
                        steps.append(unit)
                return steps

            def v_seq(xs_box):
                # each step tagged: real V units set v_frontier when emitted
                steps = []
                for c in range(L // 512):
                    steps.append(load_x(xv_d, c, xs_box, "v", xvp, "xv"))
                    for jj in range(4):
                        j = c * 4 + jj
                        for half in range(2):
                            def unit(c=c, j=j, jj=jj, half=half):
                                xs = xs_box[("v", c)]
                                wvs = wvs_box["wv"]
                                ps = kp.tile([128, 512], F32, tag="k",
                                             name=f"psv{j}{half}")
                                hs = slice(half * 512, half * 512 + 512)
                                for i in range(NI):
                                    nc.tensor.matmul(
                                        ps[:], xs[i][:, jj * 128:(jj + 1) * 128],
                                        wvs[i][:, hs],
                                        start=(i == 0), stop=(i == NI - 1))
                                dst = VP[j][:].rearrange(
                                    "p (h w) -> p h w", w=DK + 1)[
                                    :, half * 8:(half + 1) * 8, 0:DK]
                                src = ps[:].rearrange("p (h w) -> p h w", w=DK)
                                nc.gpsimd.tensor_copy(dst, src)
                                if half == 1:
                                    v_frontier[0] = j + 1
                            steps.append(unit)
                return steps

            # ---------- build filler queues ----------
            v_frontier = [0]  # number of VP tiles whose producing unit was emitted
            wvs_box = {}
            wos_box = {}
            xs_box = {}

            def load_wv():
                wvs_box["wv"] = load_w(wv_d, "v")

            def load_wo():
                wos_box["wo"] = load_w(wo_d, "o")

            v_queue = deque([load_wv] + v_seq(xs_box))

            # non-V fillers in deadline order:
            # K pass A c1..c3 (h0-h7 need m 0..3 as c chunks land), then the
            # remaining Q units (frees the wq pool slots early), K pass B
            # (m 4..7, by h8 = unit 128), wo load.
            other = deque()
            other.extend(k_seq([1, 2, 3], [0, 1, 2, 3], xs_box, "a"))  # 15
            for m in range(2, NM):
                for c in (0, 1):
                    other.append(functools.partial(q_unit, c, m))       # 12
            kB = k_seq([0, 1, 2, 3], [4, 5, 6, 7], xs_box, "b")        # 20
            other.extend(kB)
            other.append(load_wo)
            len_other = len(other)

            # ---------- direct pre-phase ----------
            # shortest path to the first exp: Q m0 (both chunks), K c0 m0
            ka0 = k_seq([0], [0, 1, 2, 3], xs_box, "a0")
            q_unit(0, 0)
            q_unit(1, 0)
            ka0[0]()     # xk c0 loads
            ka0[1]()     # K c0 m0
            # early mask loads (needed from the first mask-mul on)
            for j in range(4):
                nc.sync.dma_start(MT[j][:], mt_d.ap()[j])
            q_unit(0, 1)
            q_unit(1, 1)
            for u in ka0[2:]:
                u()
            for j in range(4, NJ):
                nc.sync.dma_start(MT[j][:], mt_d.ap()[j])
            nc.sync.dma_start(bo_sb[:], bo_d.ap())
            nc.sync.dma_start(id_sb[:], id_d.ap())

            # ---------- attention ----------
            pend_av = deque()   # (h, j, e_tile) waiting for VP[j]
            av_tiles = {}       # h -> (av0, av1)
            av_count = {}       # h -> emitted AV units (of 16)
            done_h = [0]        # heads fully AV'd + div'd

            def emit_av(h, j, e):
                av0, av1 = av_tiles[h]
                cnt = av_count[h]
                for qt in range(8):
                    avt = av0 if qt < 4 else av1
                    sl = slice(65 * (qt % 4), 65 * (qt % 4) + 65)
                    nc.tensor.matmul(
                        avt[:, sl], e[:, qt * 128:(qt + 1) * 128],
                        VP[j][:, 65 * h:65 * h + 65],
                        start=(cnt == 0 and qt % 4 == 0),
                        stop=(cnt == 15 and qt % 4 == 3),
                        skip_group_check=True)
                av_count[h] = cnt + 1

            pend_fin = deque()  # deferred transpose steps (h, qt, cq, tr)

            def finish_h(h):
                # emit all 8 divs now (DVE, off PE's critical path); defer
                # the PE transposes into later units so they never block the
                # in-order PE stream while the divs complete.
                av0, av1 = av_tiles[h]
                m, off = h // 2, DK * (h % 2)
                tr = tp.tile([128, LQ], BF, tag="t", name=f"tr{h}")
                for qt in range(8):
                    avt = av0 if qt < 4 else av1
                    sl0 = 65 * (qt % 4)
                    cq = cqp.tile([128, DK], BF, tag="cq", name=f"cq{h}_{qt}")
                    nc.vector.tensor_scalar(
                        cq[:], avt[:, sl0:sl0 + DK],
                        avt[:, sl0 + DK:sl0 + DK + 1], None,
                        op0=mybir.AluOpType.divide)
                    pend_fin.append((h, m, off, qt, cq, tr))
                del av_tiles[h]

            def pop_fin(nmax):
                for _ in range(nmax):
                    if not pend_fin:
                        return
                    h, m, off, qt, cq, tr = pend_fin.popleft()
                    nc.tensor.matmul(
                        tr[off:off + DK, qt * 128:(qt + 1) * 128],
                        cq[:], id_sb[:],
                        is_transpose=True, start=True, stop=True)
                    if qt == 7:
                        nc.gpsimd.tensor_copy(CT[m][off:off + DK, :],
                                              tr[off:off + DK, :])

            def drain_pending():
                while pend_av and pend_av[0][1] < v_frontier[0]:
                    h, j, e = pend_av.popleft()
                    emit_av(h, j, e)
                    if av_count[h] == NJ:
                        finish_h(h)
                        done_h[0] += 1

            unit_idx = [0]
            other_pops = [0]

            def pace_fillers():
                u = unit_idx[0]
                # target pops of `other`: 1/unit for first 15 (K pass A incl
                # loads), then 1 per 3 units
                target = min(len_other, u + 1 if u < 15 else 15 + (u - 15) // 3 + 1)
                while other and other_pops[0] < target:
                    other.popleft()()
                    other_pops[0] += 1
                # V: one unit per attention unit from the start (frees the
                # E-park as early as possible), plus demand-driven pulls when
                # pending AVs stack up
                if v_queue:
                    v_queue.popleft()()
                    drain_pending()
                while pend_av and len(pend_av) >= 4 and v_queue:
                    v_queue.popleft()()
                    drain_pending()

            for h in range(H):
                m, a = h // 2, h % 2
                av_tiles[h] = (
                    avp.tile([128, 260], F32, tag="av", name=f"av{h}_0"),
                    avp.tile([128, 260], F32, tag="av", name=f"av{h}_1"),
                )
                av_count[h] = 0
                for j in range(NJ):
                    s = sp.tile([128, LQ], F32, tag="s", name=f"s{h}_{j}")
                    for half in range(2):
                        lhsT = KT8[m][64 * a:64 * a + 64,
                                      j * 128:(j + 1) * 128].unsqueeze(
                            1).broadcast_to([64, 2, 128])
                        rhs = QT8[m][64 * a:64 * a + 64,
                                     half * 512:half * 512 + 512].unsqueeze(
                            1).broadcast_to([64, 2, 512])
                        nc.tensor.matmul(
                            s[:, half * 512:half * 512 + 512], lhsT, rhs,
                            start=True, stop=True, perf_mode=PM.DoubleRow)
                    e = ep.tile([128, LQ], BF, tag="e", name=f"e{h}_{j}")
                    # stride-0 DoubleRow computes 2*S, so halve the scale
                    nc.scalar.activation(e[:], s[:], AF.Exp, scale=0.0625)
                    nc.vector.tensor_mul(e[:], e[:], MT[j][:])
                    # drain BEFORE pushing the current unit: AV(h,j) is
                    # emitted at unit j+1, so the PE stream never waits on
                    # this unit's exp/mask chain (software pipelining lag 1)
                    drain_pending()
                    pop_fin(2)
                    pend_av.append((h, j, e))
                    unit_idx[0] += 1
                    pace_fillers()

            while v_queue:
                v_queue.popleft()()
            drain_pending()
            pop_fin(len(pend_fin))
            while other:
                other.popleft()()
            assert not pend_av and done_h[0] == H, (len(pend_av), done_h[0])

            # ---------- output projection ----------
            wos = wos_box["wo"]
            for t in range(NM):
                po = sp.tile([128, D], F32, tag="s", name=f"po{t}")
                for half in range(2):
                    hs = slice(half * 512, half * 512 + 512)
                    for cc in range(NI):
                        nc.tensor.matmul(
                            po[:, hs], CT[cc][:, t * 128:(t + 1) * 128],
                            wos[cc][:, hs], start=(cc == 0), stop=False)
                for half in range(2):
                    hs = slice(half * 512, half * 512 + 512)
                    nc.tensor.matmul(po[:, hs], ones_sb[:], bo_sb[:, hs],
                                     start=False, stop=True)
                f = fp.tile([128, D], BF, tag="f", name=f"f{t}")
                nc.gpsimd.tensor_copy(f[:], po[:])
                nc.sync.dma_start(out_d.ap()[t], f[:])

    nc.compile()
    nc._keep_tile_frees = keep
    return nc


@functools.lru_cache(maxsize=1)
def _built():
    return _build()


def _prep_core(c, q, k, v, mask01T, wqt, wkt, wvt, wot, bqt, bkt, bor, ident):
    b, qh = c // 2, c % 2
    qs = slice(qh * LQ, (qh + 1) * LQ)
    xq = np.ascontiguousarray(q[b, qs, :].T).astype(BF16NP).reshape(NI, 128, LQ)
    xk = np.ascontiguousarray(k[b].T).astype(BF16NP).reshape(NI, 128, L)
    xv = np.ascontiguousarray(v[b].T).astype(BF16NP).reshape(NI, 128, L)
    maskt = np.ascontiguousarray(mask01T[:, qs]).reshape(NJ, 128, LQ)
    return {
        "xq": xq, "xk": xk, "xv": xv,
        "wq": wqt, "wk": wkt, "wv": wvt, "wo": wot,
        "maskt": maskt, "bqt": bqt, "bkt": bkt, "bor": bor, "ident": ident,
    }


def kernel(q, k, v, attn_mask, Wq, bq, Wk, bk, Wv, bv, Wo, bo):
    from concourse import bass_utils

    nc = _built()

    q = np.asarray(q, np.float32)
    k = np.asarray(k, np.float32)
    v = np.asarray(v, np.float32)
    wqt = np.ascontiguousarray(np.asarray(Wq, np.float32).T).astype(BF16NP).reshape(NI, 128, D)
    wkt = np.ascontiguousarray(np.asarray(Wk, np.float32).T).astype(BF16NP).reshape(NI, 128, D)
    wvt = np.ascontiguousarray(np.asarray(Wv, np.float32).T).astype(BF16NP).reshape(NI, 128, D)
    wot = np.ascontiguousarray(np.asarray(Wo, np.float32).T).astype(BF16NP).reshape(NI, 128, D)
    mask01T = np.ascontiguousarray((np.asarray(attn_mask)[0, 0] != 0).T.astype(BF16NP))
    bqt = np.ascontiguousarray(np.asarray(bq, np.float32).reshape(NM, 128).T)
    bkt = np.ascontiguousarray(np.asarray(bk, np.float32).reshape(NM, 128).T)
    bo_eff = np.asarray(bo, np.float32) + np.asarray(Wo, np.float32) @ np.asarray(bv, np.float32)
    bor = bo_eff.astype(BF16NP).reshape(1, D)
    ident = np.eye(128, dtype=np.float32).astype(BF16NP)

    in_maps = [
        _prep_core(c, q, k, v, mask01T, wqt, wkt, wvt, wot, bqt, bkt, bor, ident)
        for c in range(NCORES)
    ]
    res = bass_utils.run_bass_kernel_spmd(nc, in_maps, core_ids=list(range(NCORES)))

    out = np.empty((B, L, D), np.float32)
    for c in range(NCORES):
        b, qh = c // 2, c % 2
        out[b, qh * LQ:(qh + 1) * LQ, :] = (
            res.results[c]["out"].astype(np.float32).reshape(LQ, D))
    return out
